# revision 65
# baseline (speedup 1.0000x reference)
"""Swin-style 3D windowed attention (B=32, N=513, C=768, H=12) on 8 TRN2 cores.

Strategy: pure data-parallel over batch (4 batches/core, no collectives).
Host does input marshalling only: bf16 casts, x transpose, the static
relative-position bias gather exp(table[idx]), and the 1/8 q-scale folded
into w_qkv.

Device pipeline (engine-balanced, CLS row/col handled out-of-loop):
  p1   qkvT[c,m] = w^T x^T (full-efficiency bf16 matmuls); PSUM->SBUF
       copies split between DVE and Act.  Interleaved per batch b:
  p1bc V of batch b transposed to natural [j, d] layout via paired-head PE
       transposes (matmul PSUM writes must be bank-aligned), ones column
       appended for free softmax sums; CLS-key score rows for all 12 heads
       via one block-diag matmul chain -> etc; CLS-query score rows
       likewise -> PE-transposed into per-column layout etqT.
  p2   per (h, b), software-pipelined A/B blocks (all tiles 512-wide):
       A: S^T j-chunk pairs, fused exp pairs on Act, exp(bias) muls split
          DVE/GpSimd; CLS-key row staged to partition 0 by DMA.
       B: PV^T accumulated directly in transposed layout [d(+sum), i]
          (CLS-query column streamed from etqT), reciprocal of the sums
          row on DVE, UNNORMALIZED output copied to aoT; recip rows go to
          DRAM with a 2-iteration lag (in-order DMA queues head-block on
          unmet deps).
  p2n  once both heads of an aoT contraction chunk finish: partition-
       broadcast DMA readback of 8 recip rows (0-stride partition APs are
       DRAM-source only) + ONE bulk [128, 2052] DVE multiply normalizes
       the whole chunk.
  p3   out = aoT^T wp + bp.
"""

import numpy as np
import ml_dtypes

PHASES = 3
KF_NORM = KF_EC1 = KF_P1BC = KF_CLS = KF_SQQ = KF_VT = True

import concourse.bass as bass
import concourse.mybir as mybir
import concourse.tile as tile
from concourse import bacc
from concourse.bass_utils import run_bass_kernel_spmd

B, N, C, H, Dh = 32, 513, 768, 12, 64
NCORES = 8
BC = B // NCORES           # 4 batches per core
M = BC * N                 # 2052 rows per core
KC = C // 128              # 6 contraction chunks
QKVC = 3 * C // 128        # 18 qkv feature chunks
JCF = 4                    # full 128-row j-chunks; CLS key j=512 is separate
NF = 512                   # full i-columns; CLS query i=512 is separate
BF16 = mybir.dt.bfloat16
F32 = mybir.dt.float32
F32R = mybir.dt.float32r
EXP = mybir.ActivationFunctionType.Exp

_nc_cache = {}


def _ceil_chunks(total, step):
    out = []
    o = 0
    while o < total:
        out.append((o, min(step, total - o)))
        o += step
    return out


M_CHUNKS = _ceil_chunks(M, 512)       # [(0,512)x4, (2048,4)]
IO_CHUNKS = _ceil_chunks(N, 512)      # [(0,512), (512,1)]
PROJ_N_CHUNKS = _ceil_chunks(C, 512)  # [(0,512), (512,256)]
PROJ_M_CHUNKS = _ceil_chunks(M, 128)


def build_bass():
    nc = bacc.Bacc(None, target_bir_lowering=False, debug=False)

    xT = nc.declare_dram_parameter("xT", [C, M], BF16, isOutput=False)
    w = nc.declare_dram_parameter("w", [C, 3 * C], BF16, isOutput=False)
    wp = nc.declare_dram_parameter("wp", [C, C], BF16, isOutput=False)
    bp = nc.declare_dram_parameter("bp", [1, C], BF16, isOutput=False)
    eb = nc.declare_dram_parameter("eb", [H, N, N], BF16, isOutput=False)
    ebc = nc.declare_dram_parameter("ebc", [H, N], BF16, isOutput=False)
    ebq = nc.declare_dram_parameter("ebq", [H, NF], BF16, isOutput=False)
    out = nc.declare_dram_parameter("out", [M, C], F32, isOutput=True)
    rec_d = nc.declare_dram_parameter("rec_d", [H * BC, N], BF16, isOutput=True)

    with tile.TileContext(nc) as tc:
        with (
            tc.tile_pool(name="persist", bufs=1) as pp,
            tc.tile_pool(name="work", bufs=3) as wk,
        ):
            # ---- persistent sbuf tensors ----
            w_sb = pp.tile([128, KC, 3 * C], BF16)
            wp_sb = pp.tile([128, KC, C], BF16)
            bp_sb = pp.tile([128, C], BF16)
            qkvT = pp.tile([128, QKVC, M], BF16)
            aoT = pp.tile([128, KC, M], BF16)
            v_nat = pp.tile([128, BC, 5, H, Dh + 1], BF16)
            ebc_sb = pp.tile([H, N], BF16)
            ebq_sb = pp.tile([H, NF], BF16)
            etc_sb = pp.tile([H, BC, N], BF16)
            etqT_sb = pp.tile([128, BC, JCF, H], BF16)
            ident_sb = pp.tile([128, 128], BF16)

            from concourse.masks import make_identity
            make_identity(nc, ident_sb[:, :])
            for b in range(BC):
                nc.vector.memset(v_nat[:, b, :, :, Dh:Dh + 1], 1.0)
            if not (KF_P1BC and KF_VT):
                for b in range(BC):
                    nc.vector.memset(v_nat[:, b, :, :, :], 1.0)
            if not (KF_P1BC and KF_CLS):
                for b in range(BC):
                    nc.vector.memset(etc_sb[:, b, :], 1.0)
            if not (KF_P1BC and KF_SQQ):
                nc.vector.memset(etqT_sb[:, :, :, :], 1.0)
            w_r = w.rearrange("(a p) n -> p a n", p=128)
            # w loaded per 128-col output slice (all 6 k-chunks at once), in
            # consumption order, so mc0's matmuls start as soon as slice 0
            # lands instead of waiting for whole-w transfers.
            nc.sync.dma_start(out=w_sb[:, :, 0:128], in_=w_r[:, :, 0:128])

            def load_eb(h):
                eb_t = wk.tile([128, JCF, NF], BF16, tag="eb", bufs=2)
                # one fused DMA: partition = j%128, then j-chunk, then i
                nc.sync.dma_start(
                    out=eb_t[:, :, :],
                    in_=bass.AP(tensor=eb, offset=h * N * N,
                                ap=[[N, 128], [128 * N, JCF], [1, NF]]),
                )
                return eb_t


            eb0_t = [None]

            with (
                tc.tile_pool(name="ps1", bufs=2, space="PSUM") as ps1,
                tc.tile_pool(name="wk1", bufs=2) as wk1,
            ):

                def cls_copies(b):
                    """Block-diag CLS stationaries for batch b: kcls, qbd."""
                    col0 = b * N
                    kcls = wk1.tile([128, KC, H], BF16, tag="kcls", bufs=2)
                    qbd = wk1.tile([128, KC, H], BF16, tag="kcls", bufs=2)
                    nc.vector.memset(kcls[:, :, :], 0.0)
                    nc.vector.memset(qbd[:, :, :], 0.0)
                    for h in range(H):
                        r0 = 64 * (h % 2)
                        nc.gpsimd.tensor_copy(
                            kcls[r0:r0 + 64, h // 2, h:h + 1],
                            qkvT[r0:r0 + 64, 6 + h // 2, col0 + 512:col0 + 513],
                        )
                        nc.gpsimd.tensor_copy(
                            qbd[r0:r0 + 64, h // 2, h:h + 1],
                            qkvT[r0:r0 + 64, h // 2, col0 + 512:col0 + 513],
                        )
                    return kcls, qbd

                def p1bc(b, kcls, qbd):
                    """Batch-b prep: CLS-key rows, CLS-query rows, V natural."""
                    col0 = b * N
                    if KF_CLS:
                        # CLS-key rows: etc[h, i] = exp(S^T[512, i]) * bias
                        stc = ps1.tile([128, N], F32, tag="stc", bufs=1)
                        for io, iw in IO_CHUNKS:
                            for kk in range(KC):
                                nc.tensor.matmul(
                                    stc[0:H, io:io + iw],
                                    kcls[:, kk, :],
                                    qkvT[:, kk, col0 + io:col0 + io + iw],
                                    start=(kk == 0), stop=(kk == KC - 1),
                                )
                        nc.scalar.activation(out=etc_sb[:, b, :],
                                             in_=stc[0:H, :], func=EXP)
                        nc.vector.tensor_mul(
                            etc_sb[:, b, :], etc_sb[:, b, :], ebc_sb[:, :]
                        )
                    if KF_SQQ:
                        # CLS-query rows: etq[h, j] = exp(S[512, j]) * bias,
                        # then PE-transpose to per-j-column layout etqT.
                        sqq = ps1.tile([128, NF], F32, tag="sqq", bufs=1)
                        for kk in range(KC):
                            nc.tensor.matmul(
                                sqq[0:H, :],
                                qbd[:, kk, :],
                                qkvT[:, 6 + kk, col0:col0 + NF],
                                start=(kk == 0), stop=(kk == KC - 1),
                            )
                        etq = wk.tile([H, NF], BF16, tag="et", bufs=2)
                        nc.scalar.activation(out=etq[:, :], in_=sqq[0:H, :],
                                             func=EXP)
                        nc.vector.tensor_mul(etq[:, :], etq[:, :],
                                             ebq_sb[:, :])
                    if KF_VT:
                        # V to natural layout, 12 heads per copy (these PE
                        # transposes also hide the etq exp/mul latency before
                        # the eqp transposes below need it)
                        for jc in range(5):
                            jo = jc * 128
                            jw = min(128, N - jo)
                            for hp in range(H // 2):
                                # both 64-row halves (heads 2hp, 2hp+1) in
                                # one transpose: full-128 input, offset-0
                                # PSUM output (matmul PSUM writes must be
                                # bank-aligned on HW)
                                vt = ps1.tile([128, 2, Dh], BF16, tag="vt",
                                              bufs=3)
                                nc.tensor.transpose(
                                    vt[:jw, :, :].rearrange("p a b -> p (a b)"),
                                    qkvT[0:128, 12 + hp,
                                         col0 + jo:col0 + jo + jw],
                                    ident_sb[0:128, :],
                                )
                                if jc % 2 == 0:
                                    nc.vector.tensor_copy(
                                        v_nat[:jw, b, jc, 2 * hp:2 * hp + 2,
                                              0:Dh],
                                        vt[:jw, :, :],
                                    )
                                else:
                                    nc.scalar.copy(
                                        v_nat[:jw, b, jc, 2 * hp:2 * hp + 2,
                                              0:Dh],
                                        vt[:jw, :, :],
                                    )
                    if KF_SQQ:
                        for jc in range(JCF):
                            eqp = ps1.tile([128, H], BF16, tag="vt", bufs=3)
                            nc.tensor.transpose(
                                eqp[:, :],
                                etq[0:H, jc * 128:(jc + 1) * 128],
                                ident_sb[0:H, 0:H],
                            )
                            nc.vector.tensor_copy(etqT_sb[:, b, jc, :],
                                                  eqp[:, :])

                # ---- phase 1: qkvT[c, m] = sum_k w[k, c] * xT[k, m] ----
                cls_tiles = None
                for mi, (mo, mw) in enumerate(M_CHUNKS):
                    xt_tiles = []
                    for kk in range(KC):
                        xt = wk1.tile([128, 512], BF16, tag="xt", bufs=6)
                        eng = nc.gpsimd if (mi == 0 or kk % 2 == 0) else nc.sync
                        eng.dma_start(
                            out=xt[:, :mw],
                            in_=xT[kk * 128:(kk + 1) * 128, mo:mo + mw],
                        )
                        xt_tiles.append(xt)
                    if mi == 0:
                        for cc in range(1, QKVC):
                            nc.sync.dma_start(
                                out=w_sb[:, :, cc * 128:(cc + 1) * 128],
                                in_=w_r[:, :, cc * 128:(cc + 1) * 128],
                            )
                    for cc in range(QKVC):
                        pt = ps1.tile([128, 512], F32, tag="mm", bufs=2)
                        for kk in range(KC):
                            nc.tensor.matmul(
                                pt[:, :mw],
                                w_sb[:, kk, cc * 128:(cc + 1) * 128],
                                xt_tiles[kk][:, :mw],
                                start=(kk == 0),
                                stop=(kk == KC - 1),
                            )
                        if cc % 2 == 0:
                            nc.vector.tensor_copy(qkvT[:, cc, mo:mo + mw], pt[:, :mw])
                        else:
                            nc.scalar.copy(qkvT[:, cc, mo:mo + mw], pt[:, :mw])
                        if cc == 11 and mi >= 1 and KF_P1BC:
                            cls_tiles = cls_copies(mi - 1)
                    if mi == 0:
                        nc.scalar.dma_start(out=ebc_sb[:, :], in_=ebc[:, :])
                        nc.scalar.dma_start(out=ebq_sb[:, :], in_=ebq[:, :])
                    if mi == 1:
                        nc.scalar.dma_start(
                            out=wp_sb[:, :, :],
                            in_=wp.rearrange("(a p) n -> p a n", p=128),
                        )
                        nc.scalar.dma_start(
                            out=bp_sb[:, :],
                            in_=bass.AP(tensor=bp, offset=0,
                                        ap=[[0, 128], [1, C]]),
                        )
                        eb0_t[0] = load_eb(0)
                    if mi >= 1 and KF_P1BC:
                        p1bc(mi - 1, *cls_tiles)

            # ---- phase 2: attention, h outer (bias reuse), b inner ----
            with tc.tile_pool(name="ps2", bufs=1, space="PSUM") as ps2:

                # deferred-normalize pipeline state: recip rows go to DRAM
                # with a 2-iteration lag (so the in-order SP queue never
                # head-blocks on an unmet dependency), then once both heads
                # of an aoT contraction chunk kk are finished, the whole
                # [128, M] chunk is normalized with ONE bulk DVE multiply
                # against a partition-broadcast DMA readback.
                recq = []   # (idx, rc) awaiting recip-row -> DRAM

                def drain_rec(nsteps):
                    if recq and len(recq) >= nsteps:
                        idx, rc_p = recq.pop(0)
                        rrow_p = rc_p[32:33, :].bitcast(BF16)[0:1, 0:N]
                        nc.sync.dma_start(out=rec_d[idx:idx + 1, :],
                                          in_=rrow_p)

                def bulk_norm(kk):
                    rcb = wk.tile([128, M], BF16, tag="rcb", bufs=1)
                    for half in range(2):
                        # rows half*64..: head 2kk+half; free dim (b, i)
                        nc.sync.dma_start(
                            out=rcb[half * 64:half * 64 + 64, :],
                            in_=bass.AP(
                                tensor=rec_d,
                                offset=(2 * kk + half) * BC * N,
                                ap=[[0, Dh], [N, BC], [1, N]]),
                        )
                    nc.vector.tensor_mul(
                        aoT[:, kk, :], aoT[:, kk, :], rcb[:, :]
                    )

                def emit_B(h, b, et, rc):
                    r0 = 64 * (h % 2)
                    col0 = b * N
                    idx = h * BC + b
                    ec1 = rc[0:1, :].bitcast(BF16)[0:1, 0:N]
                    rrow = rc[32:33, :].bitcast(BF16)[0:1, 0:N]
                    pv = ps2.tile([128, N], F32, tag="pv", bufs=2)
                    for jc in range(5):
                        lhsT = (v_nat[0:128, b, jc, h, :] if jc < JCF
                                else v_nat[0:1, b, 4, h, :])
                        rhs = (et[0:128, jc, :] if jc < JCF
                               else ec1[0:1, 0:NF])
                        nc.tensor.matmul(pv[0:Dh + 1, 0:NF], lhsT, rhs,
                                         start=(jc == 0), stop=(jc == 4))
                    for jc in range(5):
                        lhsT = (v_nat[0:128, b, jc, h, :] if jc < JCF
                                else v_nat[0:1, b, 4, h, :])
                        rhs = (etqT_sb[0:128, b, jc, h:h + 1] if jc < JCF
                               else ec1[0:1, 512:513])
                        nc.tensor.matmul(pv[0:Dh + 1, 512:513], lhsT, rhs,
                                         start=(jc == 0), stop=(jc == 4))
                    with nc.allow_low_precision(
                            reason="bf16 softmax recip, DRAM round-trip"):
                        nc.vector.reciprocal(rrow, pv[Dh:Dh + 1, :])
                    # unnormalized attention output; normalize deferred
                    nc.vector.tensor_copy(
                        aoT[r0:r0 + 64, h // 2, col0:col0 + N], pv[0:Dh, :]
                    )
                    if KF_NORM:
                        recq.append((idx, rc))
                        drain_rec(2)

                eb_t = eb0_t[0]
                prev = None
                for h in range(H if PHASES >= 2 else 0):
                    for b in range(BC):
                        r0 = 64 * (h % 2)
                        col0 = b * N
                        # A block: S^T pairs -> exp -> bias-mul
                        et = wk.tile([128, JCF, NF], BF16, tag="et", bufs=2)
                        # rc tile hosts: staged CLS-key row (p0, bf16-bitcast;
                        # matmul operands must share base partition with
                        # v_nat), recip row (p32), recip broadcast (p64:128)
                        rc = wk.tile([128, N], F32R, tag="rc", bufs=2)
                        if KF_EC1:
                            nc.sync.dma_start(
                                out=rc[0:1, :].bitcast(BF16)[0:1, 0:N],
                                in_=etc_sb[h:h + 1, b, :])
                        else:
                            nc.vector.memset(
                                rc[0:1, :].bitcast(BF16)[0:1, 0:N], 0.001)
                        # all PE S-matmuls first, then B(prev) so its DVE
                        # ops (recip/copy) are not head-blocked behind this
                        # iteration's bias-muls on the in-order DVE queue
                        sts = []
                        for half in range(2):
                            st = ps2.tile([128, 2, NF], F32, tag="st", bufs=2)
                            for j2 in range(2):
                                jo = (half * 2 + j2) * 128
                                nc.tensor.matmul(
                                    st[:, j2, :],
                                    qkvT[r0:r0 + 64, 6 + h // 2,
                                         col0 + jo:col0 + jo + 128],
                                    qkvT[r0:r0 + 64, h // 2,
                                         col0:col0 + NF],
                                    start=True, stop=True,
                                )
                            sts.append(st)
                        if b == 2 and h + 1 < H:
                            eb_next = load_eb(h + 1)
                        if prev is not None:
                            emit_B(*prev)
                        if KF_NORM and b == 3 and h >= 2 and h % 2 == 0:
                            bulk_norm(h // 2 - 1)
                        for half in range(2):
                            nc.scalar.activation(
                                out=et[:, half * 2:half * 2 + 2, :],
                                in_=sts[half][:, :, :], func=EXP,
                            )
                            if half == 0:
                                nc.gpsimd.tensor_mul(
                                    et[:, 0:2, :], et[:, 0:2, :],
                                    eb_t[:, 0:2, :],
                                )
                            else:
                                nc.vector.tensor_mul(
                                    et[:, 2:3, :], et[:, 2:3, :],
                                    eb_t[:, 2:3, :],
                                )
                                nc.vector.tensor_mul(
                                    et[:, 3:4, :], et[:, 3:4, :],
                                    eb_t[:, 3:4, :],
                                )
                        prev = (h, b, et, rc)
                    if h + 1 < H:
                        eb_t = eb_next
                if prev is not None:
                    emit_B(*prev)
                while recq:
                    drain_rec(1)
                if KF_NORM:
                    bulk_norm(KC - 1)

            # ---- phase 3: out = aoT.T @ wp + bp ----
            with (
                tc.tile_pool(name="ps3", bufs=2, space="PSUM") as ps3,
                tc.tile_pool(name="wk3", bufs=2) as wk3,
            ):
                for pmi, (mo, mw) in enumerate(PROJ_M_CHUNKS if PHASES >= 3 else PROJ_M_CHUNKS[:1]):
                    pt = ps3.tile([128, C], F32, tag="mm", bufs=3)
                    for no, nw in PROJ_N_CHUNKS:
                        for kk in range(KC):
                            nc.tensor.matmul(
                                pt[:mw, no:no + nw],
                                aoT[:, kk, mo:mo + mw],
                                wp_sb[:, kk, no:no + nw],
                                start=(kk == 0),
                                stop=(kk == KC - 1),
                            )
                    ot = wk3.tile([128, C], F32, tag="ot", bufs=2)
                    nc.vector.tensor_add(
                        ot[:mw, :], pt[:mw, :], bp_sb[:mw, :]
                    )
                    eng = nc.sync if pmi % 2 == 0 else nc.scalar
                    eng.dma_start(
                        out=out[mo:mo + mw, :], in_=ot[:mw, :]
                    )
    nc.compile()
    return nc


def _prep_inputs(x, w_qkv, w_proj, b_proj, rel_bias_table, rel_pos_index):
    bf = ml_dtypes.bfloat16
    w_host = np.asarray(w_qkv, np.float32).copy()
    w_host[:, :C] *= 0.125  # fold q scale (exact power of two)
    w_host = w_host.astype(bf)
    wp_host = np.asarray(w_proj, np.float32).astype(bf)
    bp_host = np.asarray(b_proj, np.float32).reshape(1, C).astype(bf)
    g = np.asarray(rel_bias_table, np.float32)[np.asarray(rel_pos_index)]
    eb_host = np.exp(g).transpose(2, 0, 1).copy().astype(bf)  # [H, j, i]
    ebc_host = np.ascontiguousarray(eb_host[:, 512, :])       # CLS-key row
    ebq_host = np.ascontiguousarray(eb_host[:, 0:512, 512])   # CLS-query col
    xs = np.asarray(x, np.float32).reshape(NCORES, M, C)
    in_maps = []
    for c in range(NCORES):
        xT_c = np.ascontiguousarray(xs[c].astype(bf).T)
        in_maps.append({
            "xT": xT_c, "w": w_host, "wp": wp_host, "bp": bp_host,
            "eb": eb_host, "ebc": ebc_host, "ebq": ebq_host,
        })
    return in_maps


def run(inputs, trace=False):
    if "nc" not in _nc_cache:
        _nc_cache["nc"] = build_bass()
    nc = _nc_cache["nc"]
    in_maps = _prep_inputs(**inputs)
    res = run_bass_kernel_spmd(
        nc, in_maps, core_ids=list(range(NCORES)), trace=trace
    )
    outs = [np.asarray(r["out"], np.float32).reshape(BC, N, C)
            for r in res.results]
    return np.concatenate(outs, axis=0), res


def kernel(**inputs) -> np.ndarray:
    full, _ = run(inputs, trace=False)
    return full
